# revision 8
# baseline (speedup 1.0000x reference)
"""MultiHeadAttention (Enformer-style relative attention) on 8 trn2 cores.

Sharding: core c handles batch b = c//4 and output rows [384g, 384(g+1))
with g = c%4, for ALL 8 heads. K/V/rel_k projections are duplicated across
the 4 cores of a batch; q / attention / softmax / attn@v / final embedding
are computed only for the core's 384 rows. No cross-core collective is
needed (the axon-relay ReduceScatter costs ~2.5ms, far more than the
duplicated projection work).

The device program is identical on all cores; per-core data differences are
handled host-side: xtq is the core's own 384-column slice of x^T, and pet
(positional features^T) is pre-shifted by g*384 so the rel-logit band always
starts at p0 = 1408 - 128*it.

relative_shift: per (head, i-tile) the [128, 1664] rel-logit band is written
contiguously to DRAM and read back with a skewed AP (row p starts at offset
127 - p), giving rel[p, j] = band[p, j + 127 - p]. The band is then added
into the content-logit PSUM with an identity-stationary matmul (PE) instead
of a DVE add.

attn^T for the attn@v contraction is produced by writing attn rows to DRAM
and reading back per-j-tile with xbar transpose DMAs (12 big transposes per
head instead of 36 small SBUF-SBUF ones).
"""
import math
import numpy as np
from numpy import float16 as fp16host

import concourse.bass as bass
from concourse import bacc
import concourse.mybir as mybir
import concourse.tile as tile
from concourse.bass_utils import run_bass_kernel_spmd

# problem shapes (hardcoded per contract)
B, L, D = 2, 1536, 1536
H, K, V, F = 8, 64, 192, 192
P = 128
NCORES = 8
LS = L // 4          # 384 rows per core
NIT = LS // P        # 3 i-tiles per core
NJT = L // P         # 12 j-tiles
NKT = D // P         # 12 contraction tiles
NHG = 4              # head groups (2 heads each on 128 partitions)
PE_LEN = 2 * L - 1   # 3071
PE_PAD = 2 * L       # 3072  (per-core shifted window width)
PET_FULL = PE_PAD + 3 * LS   # 4224 host-side padded feature length
BANDW = L + P        # 1664 stored band width
BCH = [(0, 512), (512, 512), (1024, 512), (1536, 128)]
CH = 512

F32 = mybir.dt.float32
F16 = mybir.dt.float16
BF16 = mybir.dt.bfloat16
LN2 = float(np.log(2.0))


# ----------------------------------------------------------------------------
# host-side constants: positional features (input-independent)
# ----------------------------------------------------------------------------

def _positional_features() -> np.ndarray:
    """Replicates reference.positional_features_all(arange(-L+1, L), F, L)."""
    pos = np.arange(-L + 1, L, dtype=np.float64)
    x = np.abs(pos)[:, None]
    f = F // 6

    max_half_life = np.log(L) / np.log(2.0)
    half_life = 2.0 ** np.linspace(3.0, max_half_life, f)
    feat_exp = np.exp(-LN2 / half_life[None, :] * x)

    widths = 2.0 ** np.arange(1, f + 1, dtype=np.float64) - 1.0
    feat_cm = (widths[None, :] > x).astype(np.float64)

    stddev = L / (2.0 * f)
    start_mean = L / f
    mean = np.linspace(start_mean, float(L), f)
    concentration = (mean / stddev) ** 2
    rate = mean / (stddev ** 2)
    safe_x = np.maximum(x, 1e-300)
    log_unnorm = (concentration[None, :] - 1.0) * np.log(safe_x) - rate[None, :] * x
    zero_x = x == 0.0
    conc_one = np.isclose(concentration[None, :] - 1.0, 0.0)
    log_unnorm = np.where(zero_x & ~conc_one, -np.inf, log_unnorm)
    log_unnorm = np.where(zero_x & conc_one, -rate[None, :] * x, log_unnorm)
    lgamma = np.vectorize(math.lgamma)
    log_norm = lgamma(concentration) - concentration * np.log(rate)
    p = np.exp(log_unnorm - log_norm[None, :]) + 1e-8
    feat_gamma = p / p.max()

    emb = np.concatenate([feat_exp, feat_cm, feat_gamma], axis=-1)
    sign = np.sign(pos)[:, None]
    emb = np.concatenate([emb, sign * emb], axis=-1)     # [3071, 192]
    return emb.astype(np.float32)


# ----------------------------------------------------------------------------
# device program (identical on all 8 cores)
# ----------------------------------------------------------------------------

def _build_nc(repeat: int = 1):
    nc = bacc.Bacc("TRN2", num_devices=NCORES, target_bir_lowering=False)

    xt_in = nc.dram_tensor("xt", [D, L], F16, kind="ExternalInput")
    xtq_in = nc.dram_tensor("xtq", [D, LS], F16, kind="ExternalInput")
    wqk_in = nc.dram_tensor("wqk", [D, 2 * H * K], F16, kind="ExternalInput")
    wv_in = nc.dram_tensor("wv", [D, H * V], F16, kind="ExternalInput")
    wrel_in = nc.dram_tensor("wrel", [2 * P, H * K], F16, kind="ExternalInput")
    pet_in = nc.dram_tensor("pet", [2 * P, PE_PAD], F16, kind="ExternalInput")
    wemb_in = nc.dram_tensor("wemb", [H * V, D], F16, kind="ExternalInput")
    qbias_in = nc.dram_tensor("qbias", [P, 2 * NHG], F32, kind="ExternalInput")
    bemb_in = nc.dram_tensor("bemb", [1, D], F32, kind="ExternalInput")
    out_t = nc.dram_tensor("out", [LS, D], F32, kind="ExternalOutput")

    def t(handle):
        return handle.tensor if hasattr(handle, "tensor") else handle

    with tile.TileContext(nc) as tc:
      for _rep in range(repeat):
        with (
            tc.tile_pool(name="consts", bufs=1) as consts,
            tc.tile_pool(name="persist", bufs=1) as persist,
            tc.tile_pool(name="band_dram", bufs=6, space="DRAM") as band_dpool,
        ):
            from concourse.masks import make_identity
            identb = consts.tile([P, P], F16)
            make_identity(nc, identb[:])
            identbb = consts.tile([P, P], BF16)
            make_identity(nc, identbb[:])

            qbias = consts.tile([P, 2 * NHG], F32)
            nc.sync.dma_start(qbias[:], qbias_in[:, :])
            bemb = consts.tile([P, D], F32)
            nc.sync.dma_start(bemb[:], bass.AP(t(bemb_in), 0, [[0, P], [1, D]]))

            # persistent projection outputs
            qcT = persist.tile([P, NHG, LS], F16)   # (q*scale + rcb)^T
            qpT = persist.tile([P, NHG, LS], F16)   # (q*scale + rpb)^T
            kT = persist.tile([P, NHG, L], F16)     # k^T
            rkT = persist.tile([P, NHG, PE_PAD], F16)
            vsb = persist.tile([P, NJT, H * V], BF16)  # v rows (j on partitions)
            outT = persist.tile([P, NKT, LS], F16)    # attn@v output^T
            sums = persist.tile([P, H * NIT], F32)
            sums3 = persist.tile([P, 8], F32)
            recip = persist.tile([P, H * NIT], F32)

            # ---------------- projections ----------------
            with (
                tc.tile_pool(name="xw", bufs=1) as xw,
                tc.tile_pool(name="pp_ps", bufs=6, space="PSUM") as pp_ps,
                tc.tile_pool(name="q_ps", bufs=2, space="PSUM") as q_ps,
            ):
                xt = xw.tile([P, NKT, L], F16)
                nc.sync.dma_start(
                    xt[:, 0:NKT // 2, :],
                    bass.AP(t(xt_in), 0, [[L, P], [P * L, NKT // 2], [1, L]]),
                )
                nc.sync.dma_start(
                    xt[:, NKT // 2:NKT, :],
                    bass.AP(t(xt_in), (NKT // 2) * P * L,
                            [[L, P], [P * L, NKT // 2], [1, L]]),
                )
                wqk = xw.tile([P, NKT, 2 * H * K], F16)
                # K-half first: k-proj is the first consumer
                nc.scalar.dma_start(
                    wqk[:, :, H * K:],
                    bass.AP(t(wqk_in), H * K,
                            [[2 * H * K, P], [P * 2 * H * K, NKT], [1, H * K]]),
                )
                nc.scalar.dma_start(
                    wqk[:, :, 0:H * K],
                    bass.AP(t(wqk_in), 0,
                            [[2 * H * K, P], [P * 2 * H * K, NKT], [1, H * K]]),
                )

                # k: full L, per head-group
                for hg in range(NHG):
                    for ch in range(L // CH):
                        ks = pp_ps.tile([P, CH], F32, tag="pps")
                        for kt in range(NKT):
                            nc.tensor.matmul(
                                ks[:],
                                wqk[:, kt, H * K + hg * P: H * K + (hg + 1) * P],
                                xt[:, kt, ch * CH:(ch + 1) * CH],
                                start=(kt == 0), stop=(kt == NKT - 1),
                            )
                        nc.vector.tensor_copy(kT[:, hg, ch * CH:(ch + 1) * CH], ks[:])

                # rel_k (wrel + pet in a short-lived scope)
                with tc.tile_pool(name="xw2", bufs=1) as xw2:
                    wrel = xw2.tile([P, 2, H * K], F16)
                    nc.scalar.dma_start(
                        wrel[:],
                        bass.AP(t(wrel_in), 0,
                                [[H * K, P], [P * H * K, 2], [1, H * K]]),
                    )
                    pet = xw2.tile([P, 2, PE_PAD], F16)
                    nc.gpsimd.dma_start(
                        pet[:],
                        bass.AP(t(pet_in), 0,
                                [[PE_PAD, P], [P * PE_PAD, 2], [1, PE_PAD]]),
                    )
                    for hg in range(NHG):
                        for ch in range(PE_PAD // CH):
                            rs = pp_ps.tile([P, CH], F32, tag="pps")
                            for kt in range(2):
                                nc.tensor.matmul(
                                    rs[:],
                                    wrel[:, kt, hg * P:(hg + 1) * P],
                                    pet[:, kt, ch * CH:(ch + 1) * CH],
                                    start=(kt == 0), stop=(kt == 1),
                                )
                            nc.vector.tensor_copy(
                                rkT[:, hg, ch * CH:(ch + 1) * CH], rs[:]
                            )

                with tc.tile_pool(name="xw3", bufs=1) as xw3:
                    xtq = xw3.tile([P, NKT, LS], F16)
                    nc.scalar.dma_start(
                        xtq[:],
                        bass.AP(t(xtq_in), 0, [[LS, P], [P * LS, NKT], [1, LS]]),
                    )
                    wv = xw3.tile([P, NKT, H * V], F16)
                    nc.gpsimd.dma_start(
                        wv[:],
                        bass.AP(t(wv_in), 0,
                                [[H * V, P], [P * H * V, NKT], [1, H * V]]),
                    )

                    # q: own rows only, per head-group
                    for hg in range(NHG):
                        qs = q_ps.tile([P, LS], F32, tag="qps")
                        for kt in range(NKT):
                            nc.tensor.matmul(
                                qs[:],
                                wqk[:, kt, hg * P:(hg + 1) * P],
                                xtq[:, kt, :],
                                start=(kt == 0), stop=(kt == NKT - 1),
                            )
                        nc.scalar.activation(
                            qcT[:, hg, :], qs[:],
                            mybir.ActivationFunctionType.Identity,
                            bias=qbias[:, 2 * hg:2 * hg + 1],
                            scale=float(K) ** -0.5,
                        )
                        nc.scalar.activation(
                            qpT[:, hg, :], qs[:],
                            mybir.ActivationFunctionType.Identity,
                            bias=qbias[:, 2 * hg + 1:2 * hg + 2],
                            scale=float(K) ** -0.5,
                        )

                    # v: full L rows, all heads
                    for j4 in range(NJT):
                        vps = []
                        for _vc in range(3):
                            vtile = pp_ps.tile([P, CH], F32, tag="pps")
                            vps.append(vtile)
                        for kt in range(NKT):
                            for vc in range(3):
                                nc.tensor.matmul(
                                    vps[vc][:],
                                    xt[:, kt, j4 * P:(j4 + 1) * P],
                                    wv[:, kt, vc * CH:(vc + 1) * CH],
                                    start=(kt == 0), stop=(kt == NKT - 1),
                                )
                        for vc in range(3):
                            nc.vector.tensor_copy(
                                vsb[:, j4, vc * CH:(vc + 1) * CH], vps[vc][:]
                            )

            # ---------------- attention ----------------
            with (
                tc.tile_pool(name="wemb_p", bufs=1) as wemb_p,
                tc.tile_pool(name="attnT_p", bufs=3) as attnT_p,
                tc.tile_pool(name="band_sb_p", bufs=4) as band_sb_p,
                tc.tile_pool(name="rel_p", bufs=4) as rel_p,
                tc.tile_pool(name="attn_p", bufs=4) as attn_p,
                tc.tile_pool(name="part_p", bufs=2) as part_p,
                tc.tile_pool(name="pc_ps", bufs=3, space="PSUM") as pc_ps,
                tc.tile_pool(name="s512_ps", bufs=3, space="PSUM") as s512_ps,
                tc.tile_pool(name="tp_ps", bufs=2, space="PSUM") as tp_ps,
            ):
                wemb = wemb_p.tile([P, NKT, D], F16)
                nc.gpsimd.dma_start(
                    wemb[:],
                    bass.AP(t(wemb_in), 0, [[D, P], [P * D, NKT], [1, D]]),
                )

                def do_attn_v(h, attnT):
                    """attn@v for a finished head."""
                    if h % 2 == 0:
                        groups = [(h * V, P), (h * V + P, K)]
                    else:
                        groups = [(h * V, K), (h * V + K, P)]
                    for hvs, m in groups:
                        base = hvs % P
                        kto = hvs // P
                        avps = s512_ps.tile([P, CH], F32, tag="s512")
                        for jt in range(NJT):
                            nc.tensor.matmul(
                                avps[base:base + m, :LS],
                                vsb[:, jt, hvs:hvs + m],
                                attnT[:, jt, :],
                                start=(jt == 0), stop=(jt == NJT - 1),
                            )
                        nc.vector.tensor_copy(
                            outT[base:base + m, kto, :], avps[base:base + m, :LS]
                        )

                prev = None
                for h in range(H):
                    hg = h // 2
                    po = (h % 2) * K
                    attnT = attnT_p.tile([P, NJT, LS], BF16, tag="attnT")

                    for it in range(NIT):
                        p0 = L - P - it * P   # band start in (pre-shifted) rkT
                        isl = slice(it * P, (it + 1) * P)

                        band_sb = band_sb_p.tile([P, BANDW], F16, tag="band")
                        for ci, (off, cw) in enumerate(BCH):
                            bps = s512_ps.tile([P, CH], F32, tag="s512")
                            nc.tensor.matmul(
                                bps[:, :cw],
                                qpT[po:po + K, hg, isl],
                                rkT[po:po + K, hg, p0 + off:p0 + off + cw],
                                start=True, stop=True,
                            )
                            nc.vector.tensor_copy(
                                band_sb[:, off:off + cw], bps[:, :cw]
                            )
                        band_dram = band_dpool.tile([P * BANDW], F16, tag="band_dram")
                        nc.sync.dma_start(
                            band_dram.rearrange("(p w) -> p w", p=P), band_sb[:]
                        )

                        # skewed read: rel[p, j] = band[p, j + 127 - p]
                        rel_sb = rel_p.tile([P, L], F16, tag="rel")
                        nc.sync.dma_start(
                            rel_sb[:],
                            bass.AP(band_dram.tensor,
                                    band_dram.offset + (P - 1),
                                    [[BANDW - 1, P], [1, L]]),
                        )

                        # content logits + identity-accumulated rel, per 512-chunk
                        col = h * NIT + it
                        attn_sb = attn_p.tile([P, L], BF16, tag="attn")
                        for ch in range(L // CH):
                            csl = slice(ch * CH, (ch + 1) * CH)
                            pcc = pc_ps.tile([P, CH], F32, tag="pc")
                            nc.tensor.matmul(
                                pcc[:],
                                qcT[po:po + K, hg, isl],
                                kT[po:po + K, hg, csl],
                                start=True, stop=False,
                            )
                            nc.tensor.matmul(
                                pcc[:],
                                identb[:],
                                rel_sb[:, csl],
                                start=False, stop=True,
                            )
                            nc.scalar.activation(
                                attn_sb[:, csl], pcc[:],
                                mybir.ActivationFunctionType.Exp,
                                accum_out=sums3[:, 3 * (col % 2) + ch:
                                                3 * (col % 2) + ch + 1],
                            )
                        nc.vector.tensor_tensor(
                            sums3[:, 6 + (col % 2):7 + (col % 2)],
                            sums3[:, 3 * (col % 2):3 * (col % 2) + 1],
                            sums3[:, 3 * (col % 2) + 1:3 * (col % 2) + 2],
                            mybir.AluOpType.add,
                        )
                        nc.vector.tensor_tensor(
                            sums[:, col:col + 1],
                            sums3[:, 6 + (col % 2):7 + (col % 2)],
                            sums3[:, 3 * (col % 2) + 2:3 * (col % 2) + 3],
                            mybir.AluOpType.add,
                        )
                        nc.vector.reciprocal(recip[:, col:col + 1], sums[:, col:col + 1])
                        nc.vector.tensor_scalar_mul(
                            attn_sb[:], attn_sb[:], recip[:, col:col + 1]
                        )
                        for jt in range(NJT):
                            tps = tp_ps.tile([P, P], BF16, tag="tp")
                            nc.tensor.transpose(
                                tps[:], attn_sb[:, jt * P:(jt + 1) * P], identbb[:]
                            )
                            if jt % 3 == 2:
                                nc.scalar.copy(
                                    attnT[:, jt, it * P:(it + 1) * P], tps[:]
                                )
                            else:
                                nc.vector.tensor_copy(
                                    attnT[:, jt, it * P:(it + 1) * P], tps[:]
                                )

                    # one-head software pipeline: previous head's attn@v runs
                    # while this head's attn writes/transposes are in flight
                    if prev is not None:
                        do_attn_v(*prev)
                    prev = (h, attnT)

                do_attn_v(*prev)

                # ---------------- final embedding ----------------
                for mi in range(NIT):
                    part = part_p.tile([P, D], F32, tag="part")
                    for nj in range(D // CH):
                        eps = s512_ps.tile([P, CH], F32, tag="s512")
                        for kt in range(NKT):
                            nc.tensor.matmul(
                                eps[:],
                                outT[:, kt, mi * P:(mi + 1) * P],
                                wemb[:, kt, nj * CH:(nj + 1) * CH],
                                start=(kt == 0), stop=(kt == NKT - 1),
                            )
                        nc.vector.tensor_tensor(
                            part[:, nj * CH:(nj + 1) * CH], eps[:],
                            bemb[:, nj * CH:(nj + 1) * CH], mybir.AluOpType.add,
                        )
                    nc.sync.dma_start(out_t[mi * P:(mi + 1) * P, :], part[:])

    nc.compile()
    return nc


_CACHE = {}


def _get_nc():
    if "nc" not in _CACHE:
        _CACHE["nc"] = _build_nc()
    return _CACHE["nc"]


def _make_in_maps(inputs, Wq, Wk, Wv, W_rel, W_emb, b_emb, rcb, rpb):
    pe = _positional_features()                      # [3071, 192]
    # device indexes rkT[w] with w = (1408 - 128*it) + x and needs
    # rel position (j - i_global + L-1) = w - g*384 -> place pe.T at
    # offset 3*LS = 1152 and slice window [1152 - g*384, +3072)
    pet_full = np.zeros((F, PET_FULL), np.float32)   # [192, 4224]
    pet_full[:, 3 * LS:3 * LS + PE_LEN] = pe.T

    xts = [np.ascontiguousarray(inputs[b].T).astype(fp16host) for b in range(B)]
    wqk = np.concatenate([Wq, Wk], axis=1).astype(fp16host)     # [D, 1024]
    wv = Wv.astype(fp16host)                                     # [D, 1536]
    wrel = np.zeros((2 * P, H * K), np.float32)
    wrel[:F] = W_rel
    wrel = wrel.astype(fp16host)
    wemb = W_emb.astype(fp16host)                                # [1536, D]
    qbias = np.empty((P, 2 * NHG), np.float32)
    for hg in range(NHG):
        qbias[:, 2 * hg] = np.concatenate([rcb[2 * hg], rcb[2 * hg + 1]])
        qbias[:, 2 * hg + 1] = np.concatenate([rpb[2 * hg], rpb[2 * hg + 1]])
    bemb = np.ascontiguousarray(b_emb.reshape(1, D))

    # per-g inputs: shifted positional features and q's x^T slice
    pets, xtqs = [], {}
    for g in range(4):
        petg = np.zeros((2 * P, PE_PAD), np.float32)
        off = 3 * LS - g * LS
        petg[:F] = pet_full[:, off: off + PE_PAD]
        pets.append(petg.astype(fp16host))
        for b in range(B):
            xtqs[(b, g)] = np.ascontiguousarray(
                np.asarray(xts[b])[:, g * LS:(g + 1) * LS]
            )

    in_maps = []
    for c in range(NCORES):
        b, g = c // 4, c % 4
        in_maps.append({
            "xt": xts[b],
            "xtq": xtqs[(b, g)],
            "wqk": wqk,
            "wv": wv,
            "wrel": wrel,
            "pet": pets[g],
            "wemb": wemb,
            "qbias": qbias,
            "bemb": bemb,
        })
    return in_maps


# ----------------------------------------------------------------------------
# entry point
# ----------------------------------------------------------------------------

def kernel(inputs, Wq, Wk, Wv, W_rel, W_emb, b_emb, rel_content_bias, rel_pos_bias):
    inputs = np.asarray(inputs, np.float32)
    Wq = np.asarray(Wq, np.float32)
    Wk = np.asarray(Wk, np.float32)
    Wv = np.asarray(Wv, np.float32)
    W_rel = np.asarray(W_rel, np.float32)
    W_emb = np.asarray(W_emb, np.float32)
    b_emb = np.asarray(b_emb, np.float32)
    rcb = np.asarray(rel_content_bias, np.float32).reshape(H, K)
    rpb = np.asarray(rel_pos_bias, np.float32).reshape(H, K)

    in_maps = _make_in_maps(inputs, Wq, Wk, Wv, W_rel, W_emb, b_emb, rcb, rpb)
    nc = _get_nc()
    res = run_bass_kernel_spmd(nc, in_maps, core_ids=list(range(NCORES)))

    out = np.empty((B, L, D), np.float32)
    for c in range(NCORES):
        b, g = c // 4, c % 4
        out[b, g * LS:(g + 1) * LS, :] = res.results[c]["out"]
    return out


# ----------------------------------------------------------------------------
# timing (not used by the grading harness; test.py calls this)
# ----------------------------------------------------------------------------

def _build_stub_nc():
    """Stub with the IDENTICAL input/output signature, near-zero compute."""
    nc = bacc.Bacc("TRN2", num_devices=NCORES, target_bir_lowering=False)
    nc.dram_tensor("xt", [D, L], F16, kind="ExternalInput")
    nc.dram_tensor("xtq", [D, LS], F16, kind="ExternalInput")
    nc.dram_tensor("wqk", [D, 2 * H * K], F16, kind="ExternalInput")
    nc.dram_tensor("wv", [D, H * V], F16, kind="ExternalInput")
    nc.dram_tensor("wrel", [2 * P, H * K], F16, kind="ExternalInput")
    nc.dram_tensor("pet", [2 * P, PE_PAD], F16, kind="ExternalInput")
    wemb_in = nc.dram_tensor("wemb", [H * V, D], F16, kind="ExternalInput")
    nc.dram_tensor("qbias", [P, 2 * NHG], F32, kind="ExternalInput")
    nc.dram_tensor("bemb", [1, D], F32, kind="ExternalInput")
    out_t = nc.dram_tensor("out", [LS, D], F32, kind="ExternalOutput")
    with tile.TileContext(nc) as tc:
        with tc.tile_pool(name="sb", bufs=1) as sb:
            t = sb.tile([P, D // 2], F16)
            nc.sync.dma_start(t[:], wemb_in[0:P, 0:D // 2])
            nc.sync.dma_start(out_t[0:P, 0:D // 4].bitcast(F16), t[:])
    nc.compile()
    return nc


def _make_timed_fn(nc, in_maps):
    """Builds a jitted shard_map callable + device-resident args."""
    import jax
    from jax.sharding import Mesh, PartitionSpec
    from jax.experimental.shard_map import shard_map
    import concourse.mybir as mybir_
    from concourse import bass2jax

    bass2jax.install_neuronx_cc_hook()
    partition_name = nc.partition_id_tensor.name if nc.partition_id_tensor else None
    in_names, out_names, out_avals, zero_outs = [], [], [], []
    for alloc in nc.m.functions[0].allocations:
        if not isinstance(alloc, mybir_.MemoryLocationSet):
            continue
        name = alloc.memorylocations[0].name
        if alloc.kind == "ExternalInput":
            if name != partition_name:
                in_names.append(name)
        elif alloc.kind == "ExternalOutput":
            shape = tuple(alloc.tensor_shape)
            dtype = mybir_.dt.np(alloc.dtype)
            out_names.append(name)
            out_avals.append(jax.core.ShapedArray(shape, dtype))
            zero_outs.append(np.zeros(shape, dtype))
    n_params = len(in_names)
    all_in_names = list(in_names) + list(out_names)
    if partition_name is not None:
        all_in_names.append(partition_name)

    def _body(*args):
        operands = list(args)
        if partition_name is not None:
            operands.append(bass2jax.partition_id_tensor())
        outs = bass2jax._bass_exec_p.bind(
            *operands,
            out_avals=tuple(out_avals),
            in_names=tuple(all_in_names),
            out_names=tuple(out_names),
            lowering_input_output_aliases=(),
            sim_require_finite=True,
            sim_require_nnan=True,
            nc=nc,
        )
        return tuple(outs)

    devices = jax.devices()[:NCORES]
    mesh = Mesh(np.asarray(devices), ("core",))
    n_outs = len(out_names)
    in_specs = (PartitionSpec("core"),) * (n_params + n_outs)
    out_specs = (PartitionSpec("core"),) * n_outs
    donate = tuple(range(n_params, n_params + n_outs))
    fn = jax.jit(
        shard_map(_body, mesh=mesh, in_specs=in_specs, out_specs=out_specs,
                  check_rep=False),
        donate_argnums=donate,
        keep_unused=True,
    )
    concat_in = [
        np.concatenate([np.asarray(in_maps[c][nm]) for c in range(NCORES)], axis=0)
        for nm in in_names
    ]
    concat_zero = [
        np.zeros((NCORES * z.shape[0], *z.shape[1:]), z.dtype) for z in zero_outs
    ]
    args = [jax.device_put(a) for a in concat_in]
    jax.block_until_ready(args)
    return fn, args, concat_zero


def _time_fn(fn, args, concat_zero, iters):
    import time as _time
    import jax
    zero_sets = []
    for _ in range(iters + 1):
        zs = [jax.device_put(z) for z in concat_zero]
        zero_sets.append(zs)
    jax.block_until_ready(zero_sets)
    outs = fn(*args, *zero_sets[-1])
    jax.block_until_ready(outs)   # warm (compile + first exec)
    t0 = _time.perf_counter()
    for i in range(iters):
        outs = fn(*args, *zero_sets[i])
        jax.block_until_ready(outs)
    t1 = _time.perf_counter()
    return (t1 - t0) / iters


def time_hw(inputs, Wq, Wk, Wv, W_rel, W_emb, b_emb, rel_content_bias,
            rel_pos_bias, iters=30):
    inputs = np.asarray(inputs, np.float32)
    rcb = np.asarray(rel_content_bias, np.float32).reshape(H, K)
    rpb = np.asarray(rel_pos_bias, np.float32).reshape(H, K)
    in_maps = _make_in_maps(
        inputs, np.asarray(Wq, np.float32), np.asarray(Wk, np.float32),
        np.asarray(Wv, np.float32), np.asarray(W_rel, np.float32),
        np.asarray(W_emb, np.float32), np.asarray(b_emb, np.float32), rcb, rpb)
    nc = _get_nc()
    fn, args, cz = _make_timed_fn(nc, in_maps)
    stub = _build_stub_nc()
    fn_s, args_s, cz_s = _make_timed_fn(stub, in_maps)

    # interleave real/stub rounds to cancel tunnel drift; median-of-rounds
    _time_fn(fn, args, cz, 2)
    _time_fn(fn_s, args_s, cz_s, 2)
    reals, stubs, diffs = [], [], []
    for _ in range(max(iters // 2, 16)):
        r = _time_fn(fn, args, cz, 5)
        s = _time_fn(fn_s, args_s, cz_s, 5)
        reals.append(r)
        stubs.append(s)
        diffs.append(r - s)   # paired: cancels slow tunnel drift
    t_real = float(np.median(reals))
    t_stub = float(np.median(stubs))
    t_diff = float(np.median(diffs))
    print(f"t_real={t_real*1e6:.1f}us t_stub={t_stub*1e6:.1f}us "
          f"pair-med={t_diff*1e6:.1f}us "
          f"(diff spread {min(diffs)*1e6:.0f}..{max(diffs)*1e6:.0f})")
    return max(t_diff, 0.0) * 1e9


# revision 9
# speedup vs baseline: 5.6526x; 5.6526x over previous
"""MultiHeadAttention (Enformer-style relative attention) on 8 trn2 cores.

Sharding: core c handles batch b = c//4 and output rows [384g, 384(g+1))
with g = c%4, for ALL 8 heads. K/V/rel_k projections are duplicated across
the 4 cores of a batch; q / attention / softmax / attn@v / final embedding
are computed only for the core's 384 rows. No cross-core collective is
needed (the axon-relay ReduceScatter costs ~2.5ms, far more than the
duplicated projection work).

The device program is identical on all cores; per-core data differences are
handled host-side: xtq is the core's own 384-column slice of x^T, and pet
(positional features^T) is pre-shifted by g*384 so the rel-logit band always
starts at p0 = 1408 - 128*it.

relative_shift: per (head, i-tile) the [128, 1664] rel-logit band is written
contiguously to DRAM and read back with a skewed AP (row p starts at offset
127 - p), giving rel[p, j] = band[p, j + 127 - p]. The band is then added
into the content-logit PSUM with an identity-stationary matmul (PE) instead
of a DVE add.

attn^T for the attn@v contraction is produced by writing attn rows to DRAM
and reading back per-j-tile with xbar transpose DMAs (12 big transposes per
head instead of 36 small SBUF-SBUF ones).
"""
import math
import numpy as np
from numpy import float16 as fp16host

import concourse.bass as bass
from concourse import bacc
import concourse.mybir as mybir
import concourse.tile as tile
from concourse.bass_utils import run_bass_kernel_spmd

# problem shapes (hardcoded per contract)
B, L, D = 2, 1536, 1536
H, K, V, F = 8, 64, 192, 192
P = 128
NCORES = 8
LS = L // 4          # 384 rows per core
NIT = LS // P        # 3 i-tiles per core
NJT = L // P         # 12 j-tiles
NKT = D // P         # 12 contraction tiles
NHG = 4              # head groups (2 heads each on 128 partitions)
PE_LEN = 2 * L - 1   # 3071
PE_PAD = 2 * L       # 3072  (per-core shifted window width)
PET_FULL = PE_PAD + 3 * LS   # 4224 host-side padded feature length
BANDW = L + P        # 1664 stored band width
BCH = [(0, 512), (512, 512), (1024, 512), (1536, 128)]
CH = 512

F32 = mybir.dt.float32
F16 = mybir.dt.float16
BF16 = mybir.dt.bfloat16
LN2 = float(np.log(2.0))


# ----------------------------------------------------------------------------
# host-side constants: positional features (input-independent)
# ----------------------------------------------------------------------------

def _positional_features() -> np.ndarray:
    """Replicates reference.positional_features_all(arange(-L+1, L), F, L)."""
    pos = np.arange(-L + 1, L, dtype=np.float64)
    x = np.abs(pos)[:, None]
    f = F // 6

    max_half_life = np.log(L) / np.log(2.0)
    half_life = 2.0 ** np.linspace(3.0, max_half_life, f)
    feat_exp = np.exp(-LN2 / half_life[None, :] * x)

    widths = 2.0 ** np.arange(1, f + 1, dtype=np.float64) - 1.0
    feat_cm = (widths[None, :] > x).astype(np.float64)

    stddev = L / (2.0 * f)
    start_mean = L / f
    mean = np.linspace(start_mean, float(L), f)
    concentration = (mean / stddev) ** 2
    rate = mean / (stddev ** 2)
    safe_x = np.maximum(x, 1e-300)
    log_unnorm = (concentration[None, :] - 1.0) * np.log(safe_x) - rate[None, :] * x
    zero_x = x == 0.0
    conc_one = np.isclose(concentration[None, :] - 1.0, 0.0)
    log_unnorm = np.where(zero_x & ~conc_one, -np.inf, log_unnorm)
    log_unnorm = np.where(zero_x & conc_one, -rate[None, :] * x, log_unnorm)
    lgamma = np.vectorize(math.lgamma)
    log_norm = lgamma(concentration) - concentration * np.log(rate)
    p = np.exp(log_unnorm - log_norm[None, :]) + 1e-8
    feat_gamma = p / p.max()

    emb = np.concatenate([feat_exp, feat_cm, feat_gamma], axis=-1)
    sign = np.sign(pos)[:, None]
    emb = np.concatenate([emb, sign * emb], axis=-1)     # [3071, 192]
    return emb.astype(np.float32)


# ----------------------------------------------------------------------------
# device program (identical on all 8 cores)
# ----------------------------------------------------------------------------

def _build_nc(repeat: int = 1):
    nc = bacc.Bacc("TRN2", num_devices=NCORES, target_bir_lowering=False)

    xt_in = nc.dram_tensor("xt", [D, L], F16, kind="ExternalInput")
    xtq_in = nc.dram_tensor("xtq", [D, LS], F16, kind="ExternalInput")
    wqk_in = nc.dram_tensor("wqk", [D, 2 * H * K], F16, kind="ExternalInput")
    wv_in = nc.dram_tensor("wv", [D, H * V], F16, kind="ExternalInput")
    wrel_in = nc.dram_tensor("wrel", [2 * P, H * K], F16, kind="ExternalInput")
    pet_in = nc.dram_tensor("pet", [2 * P, PE_PAD], F16, kind="ExternalInput")
    wemb_in = nc.dram_tensor("wemb", [H * V, D], F16, kind="ExternalInput")
    qbias_in = nc.dram_tensor("qbias", [P, 2 * NHG], F32, kind="ExternalInput")
    bemb_in = nc.dram_tensor("bemb", [1, D], F32, kind="ExternalInput")
    out_t = nc.dram_tensor("out", [LS, D], F32, kind="ExternalOutput")

    def t(handle):
        return handle.tensor if hasattr(handle, "tensor") else handle

    with tile.TileContext(nc) as tc:
      for _rep in range(repeat):
        with (
            tc.tile_pool(name="consts", bufs=1) as consts,
            tc.tile_pool(name="persist", bufs=1) as persist,
            tc.tile_pool(name="band_dram", bufs=6, space="DRAM") as band_dpool,
        ):
            from concourse.masks import make_identity
            identb = consts.tile([P, P], F16)
            make_identity(nc, identb[:])
            identbb = consts.tile([P, P], BF16)
            make_identity(nc, identbb[:])

            qbias = consts.tile([P, 2 * NHG], F32)
            nc.sync.dma_start(qbias[:], qbias_in[:, :])
            bemb = consts.tile([P, D], F32)
            nc.sync.dma_start(bemb[:], bass.AP(t(bemb_in), 0, [[0, P], [1, D]]))

            # persistent projection outputs
            qcT = persist.tile([P, NHG, LS], F16)   # (q*scale + rcb)^T
            qpT = persist.tile([P, NHG, LS], F16)   # (q*scale + rpb)^T
            kT = persist.tile([P, NHG, L], F16)     # k^T
            rkT = persist.tile([P, NHG, PE_PAD], F16)
            vsb = persist.tile([P, NJT, H * V], BF16)  # v rows (j on partitions)
            outT = persist.tile([P, NKT, LS], F16)    # attn@v output^T
            sums = persist.tile([P, H * NIT], F32)
            sums3 = persist.tile([P, 8], F32)
            recip = persist.tile([P, H * NIT], F32)

            # ---------------- projections ----------------
            with (
                tc.tile_pool(name="xw", bufs=1) as xw,
                tc.tile_pool(name="pp_ps", bufs=6, space="PSUM") as pp_ps,
                tc.tile_pool(name="q_ps", bufs=2, space="PSUM") as q_ps,
            ):
                # rel_k first: small loads, gives PE work while xt streams
                with tc.tile_pool(name="xw2", bufs=1) as xw2:
                    wrel = xw2.tile([P, 2, H * K], F16)
                    nc.sync.dma_start(
                        wrel[:],
                        bass.AP(t(wrel_in), 0,
                                [[H * K, P], [P * H * K, 2], [1, H * K]]),
                    )
                    pet = xw2.tile([P, 2, PE_PAD], F16)
                    nc.gpsimd.dma_start(
                        pet[:],
                        bass.AP(t(pet_in), 0,
                                [[PE_PAD, P], [P * PE_PAD, 2], [1, PE_PAD]]),
                    )
                    for hg in range(NHG):
                        for ch in range(PE_PAD // CH):
                            rs = pp_ps.tile([P, CH], F32, tag="pps")
                            for kt in range(2):
                                nc.tensor.matmul(
                                    rs[:],
                                    wrel[:, kt, hg * P:(hg + 1) * P],
                                    pet[:, kt, ch * CH:(ch + 1) * CH],
                                    start=(kt == 0), stop=(kt == 1),
                                )
                            nc.vector.tensor_copy(
                                rkT[:, hg, ch * CH:(ch + 1) * CH], rs[:]
                            )

                xt = xw.tile([P, NKT, L], F16)
                nc.sync.dma_start(
                    xt[:, 0:NKT // 2, :],
                    bass.AP(t(xt_in), 0, [[L, P], [P * L, NKT // 2], [1, L]]),
                )
                nc.sync.dma_start(
                    xt[:, NKT // 2:NKT, :],
                    bass.AP(t(xt_in), (NKT // 2) * P * L,
                            [[L, P], [P * L, NKT // 2], [1, L]]),
                )
                wqk = xw.tile([P, NKT, 2 * H * K], F16)
                # K-half first: k-proj is the first consumer
                nc.scalar.dma_start(
                    wqk[:, :, H * K:],
                    bass.AP(t(wqk_in), H * K,
                            [[2 * H * K, P], [P * 2 * H * K, NKT], [1, H * K]]),
                )
                nc.scalar.dma_start(
                    wqk[:, :, 0:H * K],
                    bass.AP(t(wqk_in), 0,
                            [[2 * H * K, P], [P * 2 * H * K, NKT], [1, H * K]]),
                )

                # k: full L, per head-group
                for hg in range(NHG):
                    for ch in range(L // CH):
                        ks = pp_ps.tile([P, CH], F32, tag="pps")
                        for kt in range(NKT):
                            nc.tensor.matmul(
                                ks[:],
                                wqk[:, kt, H * K + hg * P: H * K + (hg + 1) * P],
                                xt[:, kt, ch * CH:(ch + 1) * CH],
                                start=(kt == 0), stop=(kt == NKT - 1),
                            )
                        nc.vector.tensor_copy(kT[:, hg, ch * CH:(ch + 1) * CH], ks[:])

                with tc.tile_pool(name="xw3", bufs=1) as xw3:
                    xtq = xw3.tile([P, NKT, LS], F16)
                    nc.scalar.dma_start(
                        xtq[:],
                        bass.AP(t(xtq_in), 0, [[LS, P], [P * LS, NKT], [1, LS]]),
                    )
                    wv = xw3.tile([P, NKT, H * V], F16)
                    nc.gpsimd.dma_start(
                        wv[:],
                        bass.AP(t(wv_in), 0,
                                [[H * V, P], [P * H * V, NKT], [1, H * V]]),
                    )

                    # q: own rows only, per head-group
                    for hg in range(NHG):
                        qs = q_ps.tile([P, LS], F32, tag="qps")
                        for kt in range(NKT):
                            nc.tensor.matmul(
                                qs[:],
                                wqk[:, kt, hg * P:(hg + 1) * P],
                                xtq[:, kt, :],
                                start=(kt == 0), stop=(kt == NKT - 1),
                            )
                        nc.scalar.activation(
                            qcT[:, hg, :], qs[:],
                            mybir.ActivationFunctionType.Identity,
                            bias=qbias[:, 2 * hg:2 * hg + 1],
                            scale=float(K) ** -0.5,
                        )
                        nc.scalar.activation(
                            qpT[:, hg, :], qs[:],
                            mybir.ActivationFunctionType.Identity,
                            bias=qbias[:, 2 * hg + 1:2 * hg + 2],
                            scale=float(K) ** -0.5,
                        )

                    # v: full L rows, all heads
                    for j4 in range(NJT):
                        vps = []
                        for _vc in range(3):
                            vtile = pp_ps.tile([P, CH], F32, tag="pps")
                            vps.append(vtile)
                        for kt in range(NKT):
                            for vc in range(3):
                                nc.tensor.matmul(
                                    vps[vc][:],
                                    xt[:, kt, j4 * P:(j4 + 1) * P],
                                    wv[:, kt, vc * CH:(vc + 1) * CH],
                                    start=(kt == 0), stop=(kt == NKT - 1),
                                )
                        for vc in range(3):
                            nc.vector.tensor_copy(
                                vsb[:, j4, vc * CH:(vc + 1) * CH], vps[vc][:]
                            )

            # ---------------- attention ----------------
            with (
                tc.tile_pool(name="wemb_p", bufs=1) as wemb_p,
                tc.tile_pool(name="attnT_p", bufs=3) as attnT_p,
                tc.tile_pool(name="band_sb_p", bufs=4) as band_sb_p,
                tc.tile_pool(name="rel_p", bufs=4) as rel_p,
                tc.tile_pool(name="attn_p", bufs=4) as attn_p,
                tc.tile_pool(name="part_p", bufs=2) as part_p,
                tc.tile_pool(name="pc_ps", bufs=2, space="PSUM") as pc_ps,
                tc.tile_pool(name="s512_ps", bufs=3, space="PSUM") as s512_ps,
                tc.tile_pool(name="tp_ps", bufs=3, space="PSUM") as tp_ps,
            ):
                wemb = wemb_p.tile([P, NKT, D], F16)
                nc.gpsimd.dma_start(
                    wemb[:],
                    bass.AP(t(wemb_in), 0, [[D, P], [P * D, NKT], [1, D]]),
                )

                def do_attn_v(h, attnT):
                    """attn@v for a finished head."""
                    if h % 2 == 0:
                        groups = [(h * V, P), (h * V + P, K)]
                    else:
                        groups = [(h * V, K), (h * V + K, P)]
                    for hvs, m in groups:
                        base = hvs % P
                        kto = hvs // P
                        avps = s512_ps.tile([P, CH], F32, tag="s512")
                        for jt in range(NJT):
                            nc.tensor.matmul(
                                avps[base:base + m, :LS],
                                vsb[:, jt, hvs:hvs + m],
                                attnT[:, jt, :],
                                start=(jt == 0), stop=(jt == NJT - 1),
                            )
                        nc.vector.tensor_copy(
                            outT[base:base + m, kto, :], avps[base:base + m, :LS]
                        )

                prev = None
                for h in range(H):
                    hg = h // 2
                    po = (h % 2) * K
                    attnT = attnT_p.tile([P, NJT, LS], BF16, tag="attnT")

                    for it in range(NIT):
                        p0 = L - P - it * P   # band start in (pre-shifted) rkT
                        isl = slice(it * P, (it + 1) * P)

                        band_sb = band_sb_p.tile([P, BANDW], F16, tag="band")
                        for ci, (off, cw) in enumerate(BCH):
                            bps = s512_ps.tile([P, CH], F32, tag="s512")
                            nc.tensor.matmul(
                                bps[:, :cw],
                                qpT[po:po + K, hg, isl],
                                rkT[po:po + K, hg, p0 + off:p0 + off + cw],
                                start=True, stop=True,
                            )
                            nc.vector.tensor_copy(
                                band_sb[:, off:off + cw], bps[:, :cw]
                            )
                        band_dram = band_dpool.tile([P * BANDW], F16, tag="band_dram")
                        nc.sync.dma_start(
                            band_dram.rearrange("(p w) -> p w", p=P), band_sb[:]
                        )

                        # skewed read: rel[p, j] = band[p, j + 127 - p]
                        rel_sb = rel_p.tile([P, L], F16, tag="rel")
                        nc.sync.dma_start(
                            rel_sb[:],
                            bass.AP(band_dram.tensor,
                                    band_dram.offset + (P - 1),
                                    [[BANDW - 1, P], [1, L]]),
                        )

                        # content logits + identity-accumulated rel, per 512-chunk
                        col = h * NIT + it
                        attn_sb = attn_p.tile([P, L], BF16, tag="attn")
                        for ch in range(L // CH):
                            csl = slice(ch * CH, (ch + 1) * CH)
                            pcc = pc_ps.tile([P, CH], F32, tag="pc")
                            nc.tensor.matmul(
                                pcc[:],
                                qcT[po:po + K, hg, isl],
                                kT[po:po + K, hg, csl],
                                start=True, stop=False,
                            )
                            nc.tensor.matmul(
                                pcc[:],
                                identb[:],
                                rel_sb[:, csl],
                                start=False, stop=True,
                            )
                            nc.scalar.activation(
                                attn_sb[:, csl], pcc[:],
                                mybir.ActivationFunctionType.Exp,
                                accum_out=sums3[:, 3 * (col % 2) + ch:
                                                3 * (col % 2) + ch + 1],
                            )
                        nc.vector.tensor_tensor(
                            sums3[:, 6 + (col % 2):7 + (col % 2)],
                            sums3[:, 3 * (col % 2):3 * (col % 2) + 1],
                            sums3[:, 3 * (col % 2) + 1:3 * (col % 2) + 2],
                            mybir.AluOpType.add,
                        )
                        nc.vector.tensor_tensor(
                            sums[:, col:col + 1],
                            sums3[:, 6 + (col % 2):7 + (col % 2)],
                            sums3[:, 3 * (col % 2) + 2:3 * (col % 2) + 3],
                            mybir.AluOpType.add,
                        )
                        nc.vector.reciprocal(recip[:, col:col + 1], sums[:, col:col + 1])
                        nc.vector.tensor_scalar_mul(
                            attn_sb[:], attn_sb[:], recip[:, col:col + 1]
                        )
                        for jt in range(NJT):
                            tps = tp_ps.tile([P, P], BF16, tag="tp")
                            nc.tensor.transpose(
                                tps[:], attn_sb[:, jt * P:(jt + 1) * P], identbb[:]
                            )
                            if jt % 3 == 2:
                                nc.scalar.copy(
                                    attnT[:, jt, it * P:(it + 1) * P], tps[:]
                                )
                            else:
                                nc.vector.tensor_copy(
                                    attnT[:, jt, it * P:(it + 1) * P], tps[:]
                                )

                    # one-head software pipeline: previous head's attn@v runs
                    # while this head's attn writes/transposes are in flight
                    if prev is not None:
                        do_attn_v(*prev)
                    prev = (h, attnT)

                do_attn_v(*prev)

                # ---------------- final embedding ----------------
                for mi in range(NIT):
                    part = part_p.tile([P, D], F32, tag="part")
                    for nj in range(D // CH):
                        eps = s512_ps.tile([P, CH], F32, tag="s512")
                        for kt in range(NKT):
                            nc.tensor.matmul(
                                eps[:],
                                outT[:, kt, mi * P:(mi + 1) * P],
                                wemb[:, kt, nj * CH:(nj + 1) * CH],
                                start=(kt == 0), stop=(kt == NKT - 1),
                            )
                        nc.vector.tensor_tensor(
                            part[:, nj * CH:(nj + 1) * CH], eps[:],
                            bemb[:, nj * CH:(nj + 1) * CH], mybir.AluOpType.add,
                        )
                    nc.sync.dma_start(out_t[mi * P:(mi + 1) * P, :], part[:])

    nc.compile()
    return nc


_CACHE = {}


def _get_nc():
    if "nc" not in _CACHE:
        _CACHE["nc"] = _build_nc()
    return _CACHE["nc"]


def _make_in_maps(inputs, Wq, Wk, Wv, W_rel, W_emb, b_emb, rcb, rpb):
    pe = _positional_features()                      # [3071, 192]
    # device indexes rkT[w] with w = (1408 - 128*it) + x and needs
    # rel position (j - i_global + L-1) = w - g*384 -> place pe.T at
    # offset 3*LS = 1152 and slice window [1152 - g*384, +3072)
    pet_full = np.zeros((F, PET_FULL), np.float32)   # [192, 4224]
    pet_full[:, 3 * LS:3 * LS + PE_LEN] = pe.T

    xts = [np.ascontiguousarray(inputs[b].T).astype(fp16host) for b in range(B)]
    wqk = np.concatenate([Wq, Wk], axis=1).astype(fp16host)     # [D, 1024]
    wv = Wv.astype(fp16host)                                     # [D, 1536]
    wrel = np.zeros((2 * P, H * K), np.float32)
    wrel[:F] = W_rel
    wrel = wrel.astype(fp16host)
    wemb = W_emb.astype(fp16host)                                # [1536, D]
    qbias = np.empty((P, 2 * NHG), np.float32)
    for hg in range(NHG):
        qbias[:, 2 * hg] = np.concatenate([rcb[2 * hg], rcb[2 * hg + 1]])
        qbias[:, 2 * hg + 1] = np.concatenate([rpb[2 * hg], rpb[2 * hg + 1]])
    bemb = np.ascontiguousarray(b_emb.reshape(1, D))

    # per-g inputs: shifted positional features and q's x^T slice
    pets, xtqs = [], {}
    for g in range(4):
        petg = np.zeros((2 * P, PE_PAD), np.float32)
        off = 3 * LS - g * LS
        petg[:F] = pet_full[:, off: off + PE_PAD]
        pets.append(petg.astype(fp16host))
        for b in range(B):
            xtqs[(b, g)] = np.ascontiguousarray(
                np.asarray(xts[b])[:, g * LS:(g + 1) * LS]
            )

    in_maps = []
    for c in range(NCORES):
        b, g = c // 4, c % 4
        in_maps.append({
            "xt": xts[b],
            "xtq": xtqs[(b, g)],
            "wqk": wqk,
            "wv": wv,
            "wrel": wrel,
            "pet": pets[g],
            "wemb": wemb,
            "qbias": qbias,
            "bemb": bemb,
        })
    return in_maps


# ----------------------------------------------------------------------------
# entry point
# ----------------------------------------------------------------------------

def kernel(inputs, Wq, Wk, Wv, W_rel, W_emb, b_emb, rel_content_bias, rel_pos_bias):
    inputs = np.asarray(inputs, np.float32)
    Wq = np.asarray(Wq, np.float32)
    Wk = np.asarray(Wk, np.float32)
    Wv = np.asarray(Wv, np.float32)
    W_rel = np.asarray(W_rel, np.float32)
    W_emb = np.asarray(W_emb, np.float32)
    b_emb = np.asarray(b_emb, np.float32)
    rcb = np.asarray(rel_content_bias, np.float32).reshape(H, K)
    rpb = np.asarray(rel_pos_bias, np.float32).reshape(H, K)

    in_maps = _make_in_maps(inputs, Wq, Wk, Wv, W_rel, W_emb, b_emb, rcb, rpb)
    nc = _get_nc()
    res = run_bass_kernel_spmd(nc, in_maps, core_ids=list(range(NCORES)))

    out = np.empty((B, L, D), np.float32)
    for c in range(NCORES):
        b, g = c // 4, c % 4
        out[b, g * LS:(g + 1) * LS, :] = res.results[c]["out"]
    return out


# ----------------------------------------------------------------------------
# timing (not used by the grading harness; test.py calls this)
# ----------------------------------------------------------------------------

def _build_stub_nc():
    """Stub with the IDENTICAL input/output signature, near-zero compute."""
    nc = bacc.Bacc("TRN2", num_devices=NCORES, target_bir_lowering=False)
    nc.dram_tensor("xt", [D, L], F16, kind="ExternalInput")
    nc.dram_tensor("xtq", [D, LS], F16, kind="ExternalInput")
    nc.dram_tensor("wqk", [D, 2 * H * K], F16, kind="ExternalInput")
    nc.dram_tensor("wv", [D, H * V], F16, kind="ExternalInput")
    nc.dram_tensor("wrel", [2 * P, H * K], F16, kind="ExternalInput")
    nc.dram_tensor("pet", [2 * P, PE_PAD], F16, kind="ExternalInput")
    wemb_in = nc.dram_tensor("wemb", [H * V, D], F16, kind="ExternalInput")
    nc.dram_tensor("qbias", [P, 2 * NHG], F32, kind="ExternalInput")
    nc.dram_tensor("bemb", [1, D], F32, kind="ExternalInput")
    out_t = nc.dram_tensor("out", [LS, D], F32, kind="ExternalOutput")
    with tile.TileContext(nc) as tc:
        with tc.tile_pool(name="sb", bufs=1) as sb:
            t = sb.tile([P, D // 2], F16)
            nc.sync.dma_start(t[:], wemb_in[0:P, 0:D // 2])
            nc.sync.dma_start(out_t[0:P, 0:D // 4].bitcast(F16), t[:])
    nc.compile()
    return nc


def _make_timed_fn(nc, in_maps):
    """Builds a jitted shard_map callable + device-resident args."""
    import jax
    from jax.sharding import Mesh, PartitionSpec
    from jax.experimental.shard_map import shard_map
    import concourse.mybir as mybir_
    from concourse import bass2jax

    bass2jax.install_neuronx_cc_hook()
    partition_name = nc.partition_id_tensor.name if nc.partition_id_tensor else None
    in_names, out_names, out_avals, zero_outs = [], [], [], []
    for alloc in nc.m.functions[0].allocations:
        if not isinstance(alloc, mybir_.MemoryLocationSet):
            continue
        name = alloc.memorylocations[0].name
        if alloc.kind == "ExternalInput":
            if name != partition_name:
                in_names.append(name)
        elif alloc.kind == "ExternalOutput":
            shape = tuple(alloc.tensor_shape)
            dtype = mybir_.dt.np(alloc.dtype)
            out_names.append(name)
            out_avals.append(jax.core.ShapedArray(shape, dtype))
            zero_outs.append(np.zeros(shape, dtype))
    n_params = len(in_names)
    all_in_names = list(in_names) + list(out_names)
    if partition_name is not None:
        all_in_names.append(partition_name)

    def _body(*args):
        operands = list(args)
        if partition_name is not None:
            operands.append(bass2jax.partition_id_tensor())
        outs = bass2jax._bass_exec_p.bind(
            *operands,
            out_avals=tuple(out_avals),
            in_names=tuple(all_in_names),
            out_names=tuple(out_names),
            lowering_input_output_aliases=(),
            sim_require_finite=True,
            sim_require_nnan=True,
            nc=nc,
        )
        return tuple(outs)

    devices = jax.devices()[:NCORES]
    mesh = Mesh(np.asarray(devices), ("core",))
    n_outs = len(out_names)
    in_specs = (PartitionSpec("core"),) * (n_params + n_outs)
    out_specs = (PartitionSpec("core"),) * n_outs
    donate = tuple(range(n_params, n_params + n_outs))
    fn = jax.jit(
        shard_map(_body, mesh=mesh, in_specs=in_specs, out_specs=out_specs,
                  check_rep=False),
        donate_argnums=donate,
        keep_unused=True,
    )
    concat_in = [
        np.concatenate([np.asarray(in_maps[c][nm]) for c in range(NCORES)], axis=0)
        for nm in in_names
    ]
    concat_zero = [
        np.zeros((NCORES * z.shape[0], *z.shape[1:]), z.dtype) for z in zero_outs
    ]
    args = [jax.device_put(a) for a in concat_in]
    jax.block_until_ready(args)
    return fn, args, concat_zero


def _time_fn(fn, args, concat_zero, iters):
    import time as _time
    import jax
    zero_sets = []
    for _ in range(iters + 1):
        zs = [jax.device_put(z) for z in concat_zero]
        zero_sets.append(zs)
    jax.block_until_ready(zero_sets)
    outs = fn(*args, *zero_sets[-1])
    jax.block_until_ready(outs)   # warm (compile + first exec)
    t0 = _time.perf_counter()
    for i in range(iters):
        outs = fn(*args, *zero_sets[i])
        jax.block_until_ready(outs)
    t1 = _time.perf_counter()
    return (t1 - t0) / iters


def time_hw(inputs, Wq, Wk, Wv, W_rel, W_emb, b_emb, rel_content_bias,
            rel_pos_bias, iters=30):
    inputs = np.asarray(inputs, np.float32)
    rcb = np.asarray(rel_content_bias, np.float32).reshape(H, K)
    rpb = np.asarray(rel_pos_bias, np.float32).reshape(H, K)
    in_maps = _make_in_maps(
        inputs, np.asarray(Wq, np.float32), np.asarray(Wk, np.float32),
        np.asarray(Wv, np.float32), np.asarray(W_rel, np.float32),
        np.asarray(W_emb, np.float32), np.asarray(b_emb, np.float32), rcb, rpb)
    nc = _get_nc()
    fn, args, cz = _make_timed_fn(nc, in_maps)
    stub = _build_stub_nc()
    fn_s, args_s, cz_s = _make_timed_fn(stub, in_maps)

    # interleave real/stub rounds to cancel tunnel drift; median-of-rounds
    _time_fn(fn, args, cz, 2)
    _time_fn(fn_s, args_s, cz_s, 2)
    reals, stubs, diffs = [], [], []
    for _ in range(max(iters // 2, 16)):
        r = _time_fn(fn, args, cz, 5)
        s = _time_fn(fn_s, args_s, cz_s, 5)
        reals.append(r)
        stubs.append(s)
        diffs.append(r - s)   # paired: cancels slow tunnel drift
    t_real = float(np.median(reals))
    t_stub = float(np.median(stubs))
    t_diff = float(np.median(diffs))
    print(f"t_real={t_real*1e6:.1f}us t_stub={t_stub*1e6:.1f}us "
          f"pair-med={t_diff*1e6:.1f}us "
          f"(diff spread {min(diffs)*1e6:.0f}..{max(diffs)*1e6:.0f})")
    return max(t_diff, 0.0) * 1e9
